# revision 29
# baseline (speedup 1.0000x reference)
"""MiMo audio attention (B=2, S=2048, H=2048, NH=16, NKV=4, HD=128) on 8 trn2 cores.

Sharding: TP over heads x DP over batch. Cores 0-3 own batch 0, cores 4-7 own
batch 1. Within a batch group, TP rank t owns query heads [4t, 4t+4) and KV
head t (GQA: q head g uses kv head g//4, so the 4 q heads of rank t all use kv
head t). Each core computes a full-width o_proj partial over its 512
attn-output features; the host sums the 4 partials per batch group (the
"all-reduce after o_proj" of the TP scheme, done at gather time).

Device layout strategy (per core):
  - everything is fp16 on SBUF/DRAM (PSUM accumulation stays fp32): matmuls
    run at the same 1 cycle/row as fp32r at ANY free size (no >=256-col
    constraint), DVE tensor ops get the 2x two-byte fast path, and all DMA
    bytes halve. fp16 quantization (~5e-4 relative) is far inside the 2e-2
    correctness envelope.
  - hidden is fed pre-transposed as hidT [H, S] so the QKV projections run
    with W as the stationary operand and produce Q^T/K^T/V^T [feat, tok].
  - RoPE is applied in the [feat, tok] layout: cos/sin tables [128, S] are
    host-precomputed; rotate_half becomes a 64-partition swap done with two
    SBUF->SBUF DMAs.
  - scores are computed transposed, S^T[k, q] = K^T_tile^T @ Q^T, so attn@V
    needs no transposes: out^T[d, q] = V_tile^T @ exp(S^T). The softmax
    denominator is NOT a per-tile ones-matmul: masked exp tiles are
    accumulated elementwise (DVE/Pool alternating) into a per-(head,q-tile)
    fp16 accumulator, and a single [128,128] ones-matmul per (h,j) reduces it
    across partitions. This removes ~13% of all TensorE work.
  - softmax uses no max-subtraction (scores are O(5); exp is safe) and the
    causal mask is a multiplicative [128,128] (c>=r) triangle applied only to
    diagonal 128-blocks, post-exp, with exact 128-granular column truncation.
"""

import numpy as np

import concourse.bass as bass
import concourse.mybir as mybir
import concourse.tile as tile
from concourse import bacc, bass_utils
from concourse.tile_rust import add_dep_helper

B, S, H = 2, 2048, 2048
NH, NKV, HD = 16, 4, 128
THETA = 10000.0
SCALE = HD ** -0.5

NCORES = 8
TP = 4                 # cores per batch group
HPC = NH // TP         # 4 query heads per core
KT = H // 128          # 16 contraction tiles for projections
TT = S // 512          # 4 token tiles of 512
ST = S // 128          # 16 token tiles of 128

F32 = mybir.dt.float32
F16 = mybir.dt.float16
AF = mybir.ActivationFunctionType

_PROGRAM_CACHE = {}


def build_program(npasses=1):
    key = ("nc", npasses)
    if key in _PROGRAM_CACHE:
        return _PROGRAM_CACHE[key]

    nc = bacc.Bacc("TRN2", target_bir_lowering=False, debug=False, num_devices=NCORES)

    hidT = nc.declare_dram_parameter("hidT", [H, S], F16, isOutput=False)
    # weights arrive host-packed partition-major ([128, KT*cols]) so every
    # weight DMA is a >=2KB-contiguous run (256B runs pay a 2x DMA latency
    # penalty)
    wq = nc.declare_dram_parameter("wq", [128, KT * HPC * HD], F16, isOutput=False)
    wk = nc.declare_dram_parameter("wk", [128, KT * HD], F16, isOutput=False)
    wv = nc.declare_dram_parameter("wv", [128, KT * HD], F16, isOutput=False)
    wo = nc.declare_dram_parameter("wo", [HPC * HD, H], F16, isOutput=False)
    bq = nc.declare_dram_parameter("bq", [HD, HPC], F32, isOutput=False)
    bk = nc.declare_dram_parameter("bk", [HD, 1], F32, isOutput=False)
    bv = nc.declare_dram_parameter("bv", [HD, 1], F32, isOutput=False)
    cosT = nc.declare_dram_parameter("cosT", [HD, S], F16, isOutput=False)
    sinT = nc.declare_dram_parameter("sinT", [HD, S], F16, isOutput=False)
    mask = nc.declare_dram_parameter("mask", [128, 128], F16, isOutput=False)
    ones = nc.declare_dram_parameter("ones", [128, 128], F16, isOutput=False)
    eye = nc.declare_dram_parameter("eye", [128, 128], F16, isOutput=False)
    out_d = nc.declare_dram_parameter("out", [S, H], F16, isOutput=True)

    hidT_r = hidT.ap().rearrange("(a p) m -> p a m", p=128)   # [128, 16, 2048]
    wq_r = wq.ap().rearrange("p (a m) -> p a m", a=KT)
    wk_r = wk.ap().rearrange("p (a m) -> p a m", a=KT)
    wv_r = wv.ap().rearrange("p (a m) -> p a m", a=KT)

    with tile.TileContext(nc) as tc:
        with (
            tc.tile_pool(name="consts", bufs=1) as consts,
            tc.tile_pool(name="persist", bufs=1) as persist,
            tc.tile_pool(name="wts", bufs=1) as wts,
            tc.tile_pool(name="hid3", bufs=1) as hid3,
            tc.tile_pool(name="vtr", bufs=2) as vtrp,
            tc.tile_pool(name="expp", bufs=6) as expp,
            tc.tile_pool(name="accp", bufs=4) as accp,
            tc.tile_pool(name="recp", bufs=2) as recp,
            tc.tile_pool(name="trig", bufs=1) as trig,
            tc.tile_pool(name="stage", bufs=3) as stage,
        ):
            mask_sb = consts.tile([128, 128], F16)
            ones_sb = consts.tile([128, 128], F16)
            eye_sb = consts.tile([128, 128], F16)
            bq_sb = consts.tile([HD, HPC], F32)
            bk_sb = consts.tile([HD, 1], F32)
            bv_sb = consts.tile([HD, 1], F32)
            # const loads ride the idle Pool queue so the ACT queue's first DMAs
            # are the weight chunks the first matmuls wait on
            nc.gpsimd.dma_start(bq_sb[:], bq.ap())
            nc.gpsimd.dma_start(bk_sb[:], bk.ap())
            nc.gpsimd.dma_start(bv_sb[:], bv.ap())
            nc.gpsimd.dma_start(eye_sb[:], eye.ap())
            nc.gpsimd.dma_start(mask_sb[:], mask.ap())
            nc.gpsimd.dma_start(ones_sb[:], ones.ap())

            def emit(pid):
                # persistent activations, one tile per (tensor, tok-tile) so
                # cross-phase dependencies stay precise
                qt_sb = [[persist.tile([128, 512], F16, name=f"qt{h}_{t}", tag=f"qt{h}_{t}")
                          for t in range(TT)] for h in range(HPC)]
                kt_sb = [persist.tile([128, 512], F16, name=f"kt_{t}", tag=f"kt_{t}") for t in range(TT)]
                v_sb = [persist.tile([128, 128], F16, name=f"v_{i}", tag=f"v_{i}") for i in range(ST)]
                ao_sb = [[persist.tile([128, 512], F16, name=f"ao{h}_{t}", tag=f"ao{h}_{t}")
                          for t in range(TT)] for h in range(HPC)]

                cos_sb = trig.tile([HD, S], F16, name="cos_sb")
                sin_sb = trig.tile([HD, S], F16, name="sin_sb")

                def rope_inplace(t, dst, after=None):
                    """dst holds raw (biased) values for tok tile t; rotate in place.

                    `after`: optional instruction; adds a scheduler-only edge so
                    this chain is ordered behind it (keeps the in-order DVE FIFO
                    from blocking earlier-needed work behind this chain)."""
                    tok = bass.ds(t * 512, 512)
                    swp = stage.tile([128, 512], F16, tag="swp")
                    d0 = nc.gpsimd.dma_start(swp[0:64, :], dst[64:128, :])
                    d1 = nc.gpsimd.dma_start(swp[64:128, :], dst[0:64, :])
                    if after is not None:
                        add_dep_helper(d0.ins, after.ins, False, "delay last-tok rope")
                        add_dep_helper(d1.ins, after.ins, False, "delay last-tok rope")
                    m0 = nc.vector.tensor_mul(dst[:], dst[:], cos_sb[:, tok])
                    if after is not None:
                        add_dep_helper(m0.ins, after.ins, False, "delay last-tok rope")
                    nc.vector.tensor_mul(swp[:], swp[:], sin_sb[:, tok])
                    nc.vector.tensor_add(dst[:], dst[:], swp[:])

                # ---------------- phase 1: QKV projection + RoPE -----------------
                # Only tok tiles 0..TT-2 project here; the t=TT-1 projection is
                # deferred into the attention phase as PE filler work (see
                # phase 2), where attention's exp-rate otherwise leaves the
                # tensor engine idle.
                with (
                    tc.tile_pool(name=f"hidp{pid}", bufs=2) as hidp,
                    tc.tile_pool(name=f"ppsA{pid}", bufs=1, space=bass.MemorySpace.PSUM) as pps,
                    tc.tile_pool(name=f"ppsB{pid}", bufs=1, space=bass.MemorySpace.PSUM) as ppsB,
                ):
                    wq_sb = wts.tile([128, KT, HPC * HD], F16)
                    wk_sb = wts.tile([128, KT, HD], F16)
                    wv_sb = wts.tile([128, KT, HD], F16)

                    vtr_tiles = {}
                    for t in range(TT - 1):
                        tok = bass.ds(t * 512, 512)
                        q_ps = [pps.tile([128, 512], F32, name=f"qps{f}", tag=f"qps{f}") for f in range(HPC)]
                        k_ps = ppsB.tile([128, 512], F32, tag="kps")
                        v_ps = ppsB.tile([128, 512], F32, tag="vps")
                        # V^T -> V transposes for the PREVIOUS tok tile, emitted
                        # first (their inputs are long ready; copies go to ACT so
                        # they never queue behind RoPE work on the DVE)
                        if t > 0:
                            for i in range(4 * (t - 1), 4 * t):
                                tp = ppsB.tile([128, 128], F16, tag="vt", bufs=2)
                                nc.tensor.transpose(tp[:], vtr_tiles[t - 1][:, (i % 4) * 128:(i % 4 + 1) * 128], eye_sb[:])
                                nc.scalar.activation(v_sb[i][:], tp[:], AF.Identity)
                        for kc in range(KT // 4):       # 4 k-slices per DMA chunk
                            if t == 0:
                                nc.scalar.dma_start(wk_sb[:, 4 * kc:4 * (kc + 1), :],
                                                    wk_r[:, 4 * kc:4 * (kc + 1), :])
                                if kc == 0:
                                    # split the first wq chunk per head slice so
                                    # the first Q matmul starts after 128KB, not
                                    # 512KB, and before wv occupies the HWDGE
                                    for f in range(HPC):
                                        nc.scalar.dma_start(
                                            wq_sb[:, 0:4, f * 128:(f + 1) * 128],
                                            wq_r[:, 0:4, f * 128:(f + 1) * 128])
                                    nc.scalar.dma_start(wv_sb[:, 0:4, :], wv_r[:, 0:4, :])
                                else:
                                    nc.scalar.dma_start(wv_sb[:, 4 * kc:4 * (kc + 1), :],
                                                        wv_r[:, 4 * kc:4 * (kc + 1), :])
                                    nc.scalar.dma_start(wq_sb[:, 4 * kc:4 * (kc + 1), :],
                                                        wq_r[:, 4 * kc:4 * (kc + 1), :])
                            ht = hidp.tile([128, 4, 512], F16)
                            if t == 0 and kc == 0:
                                # split the first chunk so the first matmuls
                                # start after half the transfer
                                nc.sync.dma_start(ht[:, 0:2, :], hidT_r[:, 0:2, tok])
                                nc.sync.dma_start(ht[:, 2:4, :], hidT_r[:, 2:4, tok])
                            else:
                                nc.sync.dma_start(ht[:], hidT_r[:, 4 * kc:4 * (kc + 1), tok])
                            for kk in range(4):
                                k = 4 * kc + kk
                                st, sp = (k == 0), (k == KT - 1)
                                nc.tensor.matmul(k_ps[:], wk_sb[:, k, :], ht[:, kk, :], start=st, stop=sp)
                                for f in range(HPC):
                                    nc.tensor.matmul(q_ps[f][:], wq_sb[:, k, f * 128:(f + 1) * 128], ht[:, kk, :], start=st, stop=sp)
                                nc.tensor.matmul(v_ps[:], wv_sb[:, k, :], ht[:, kk, :], start=st, stop=sp)
                        if t == 0:
                            nc.scalar.dma_start(cos_sb[:], cosT.ap())
                            nc.scalar.dma_start(sin_sb[:], sinT.ap())

                        # pass 1: evacuate all six PSUM banks. Mid-phase tiles
                        # go all-ACT (keeps DVE free for RoPE); the LAST tile's
                        # six evacs all land after its final matmul and gate the
                        # phase-2 PSUM pool handover, so split them ACT/DVE to
                        # halve the serial drain.
                        last_t = (t == TT - 2)
                        for f in range(HPC):
                            if last_t and f % 2 == 1:
                                nc.vector.tensor_scalar_add(qt_sb[f][t][:], q_ps[f][:], bq_sb[:, f:f + 1])
                            else:
                                nc.scalar.activation(qt_sb[f][t][:], q_ps[f][:], AF.Identity, bias=bq_sb[:, f:f + 1])
                        vtr = vtrp.tile([128, 512], F16, tag="vtr")
                        vtr_tiles[t] = vtr
                        nc.scalar.activation(vtr[:], v_ps[:], AF.Identity, bias=bv_sb[:])
                        if last_t:
                            nc.vector.tensor_scalar_add(kt_sb[t][:], k_ps[:], bk_sb[:])
                        else:
                            nc.scalar.activation(kt_sb[t][:], k_ps[:], AF.Identity, bias=bk_sb[:])

                        # pass 2: RoPE in place on Q heads and K
                        for f in range(HPC):
                            rope_inplace(t, qt_sb[f][t])
                        rope_inplace(t, kt_sb[t])

                # ---------------- phase 2: attention + o_proj ---------------------
                # Attention's inner loop is ACT-bound (one exp per 128-k tile at
                # ~680ns vs ~340ns of PE work), so PE-side FILLER work drains
                # into the attention instruction stream: first the deferred
                # t=TT-1 projection (one PSUM bank, one output at a time), then
                # o_proj 512-col chunks as their ao strips complete. Supply
                # matches demand: proj units cover j0+j1, strip0 covers j2,
                # strips 1-2 cover j3, strip3 is the PE-only tail.
                with (
                    tc.tile_pool(name=f"wo_p{pid}", bufs=1) as wo_p,
                    tc.tile_pool(name=f"outp{pid}", bufs=3) as outp,
                    tc.tile_pool(name=f"scps{pid}", bufs=2, space=bass.MemorySpace.PSUM) as scps,
                    tc.tile_pool(name=f"oups{pid}", bufs=2, space=bass.MemorySpace.PSUM) as oups,
                    tc.tile_pool(name=f"smps{pid}", bufs=1, space=bass.MemorySpace.PSUM) as smps,
                    tc.tile_pool(name=f"opps{pid}", bufs=2, space=bass.MemorySpace.PSUM) as opps,
                    tc.tile_pool(name=f"prps{pid}", bufs=1, space=bass.MemorySpace.PSUM) as prps,
                ):
                    t3 = TT - 1
                    # t=3 hidden chunks + wo ride the now-idle SP queue
                    ht3 = []
                    for kc in range(KT // 4):
                        h3 = hid3.tile([128, 4, 512], F16, tag=f"t3c{kc}")
                        nc.sync.dma_start(h3[:], hidT_r[:, 4 * kc:4 * (kc + 1), bass.ds(t3 * 512, 512)])
                        ht3.append(h3)
                    wo_sb = wo_p.tile([128, HPC, H], F16)
                    for k in range(HPC):
                        nc.sync.dma_start(wo_sb[:, k, :],
                                          wo.ap().rearrange("(t p) m -> p t m", p=128)[:, k, :])

                    def attn_tile(h, j, drain=None):
                        ou_ps = oups.tile([128, 512], F32, tag="ou")
                        acc = accp.tile([128, 512], F16, tag="acc")
                        last = 4 * j + 3
                        pend = None  # software-pipeline: consumer MMs trail by one i
                        for i in range(last + 1):
                            d = i - 4 * j
                            c0 = 0 if d < 0 else 128 * d
                            sc_ps = scps.tile([128, 512], F32, tag="sc")
                            nc.tensor.matmul(
                                sc_ps[:, c0:512],
                                kt_sb[i // 4][:, (i % 4) * 128:(i % 4 + 1) * 128],
                                qt_sb[h][j][:, c0:512],
                                start=True, stop=True,
                            )
                            ex = expp.tile([128, 512], F16)
                            nc.scalar.activation(ex[:, c0:512], sc_ps[:, c0:512], AF.Exp, scale=SCALE)
                            if d >= 0:
                                # causal mask on the diagonal 128-block
                                nc.vector.tensor_mul(
                                    ex[:, c0:c0 + 128],
                                    ex[:, c0:c0 + 128],
                                    mask_sb[:],
                                )
                            # accumulate exp for the softmax denominator
                            if i == 0:
                                nc.vector.tensor_copy(acc[:], ex[:])
                            else:
                                nc.vector.tensor_add(acc[:, c0:512], acc[:, c0:512], ex[:, c0:512])
                            if pend is not None:
                                pex, pc0, pi = pend
                                nc.tensor.matmul(ou_ps[:, pc0:512], v_sb[pi][:], pex[:, pc0:512],
                                                 start=(pi == 0), stop=False)
                            pend = (ex, c0, i)
                            if drain is not None and i % 2 == 1:
                                drain(1)
                        pex, pc0, pi = pend
                        nc.tensor.matmul(ou_ps[:, pc0:512], v_sb[pi][:], pex[:, pc0:512],
                                         start=(pi == 0), stop=True)
                        sm_ps = smps.tile([128, 512], F32, tag="sm")
                        nc.tensor.matmul(sm_ps[:], ones_sb[:], acc[:], start=True, stop=True)
                        rec = recp.tile([128, 512], F32)
                        nc.vector.reciprocal_approx_fast(rec[:], sm_ps[:])
                        return nc.vector.tensor_mul(ao_sb[h][j][:], ou_ps[:], rec[:])

                    # --- filler machinery: closures drained into attention ---
                    from collections import deque
                    filler = deque()

                    def emit_chunk(m, n, alt=False):
                        ot = outp.tile([128, 512], F16, tag="ot")
                        ps = opps.tile([128, 512], F32, tag="op")
                        for k in range(HPC):
                            nc.tensor.matmul(
                                ps[:],
                                ao_sb[k][m // 4][:, (m % 4) * 128:(m % 4 + 1) * 128],
                                wo_sb[:, k, n * 512:(n + 1) * 512],
                                start=(k == 0), stop=(k == HPC - 1),
                            )
                        # during attention the ACT queue must stay exp-only, so
                        # filler evacuations go to DVE; tail chunks alternate
                        # engines and output-DMA queues (SP/ACT) to halve the
                        # end-of-kernel queue drain
                        if alt and (m + n) % 2 == 0:
                            nc.scalar.activation(ot[:], ps[:], AF.Identity)
                        else:
                            nc.vector.tensor_copy(ot[:], ps[:])
                        nc.sync.dma_start(out_d.ap()[m * 128:(m + 1) * 128, n * 512:(n + 1) * 512],
                                          ot[:])

                    def drain(k, alt=False):
                        for _ in range(k):
                            if filler:
                                filler.popleft()(alt)

                    def queue_strip(j):
                        for m in range(4 * j, 4 * j + 4):
                            for n in range(TT):
                                filler.append(lambda alt, m=m, n=n: emit_chunk(m, n, alt))

                    def queue_vtrans(t):
                        for i in range(4 * t, 4 * t + 4):
                            def unit(alt, i=i, t=t):
                                tp = opps.tile([128, 128], F16, name="vtp", tag="op")
                                nc.tensor.transpose(tp[:], vtr_tiles[t][:, (i % 4) * 128:(i % 4 + 1) * 128], eye_sb[:])
                                nc.scalar.activation(v_sb[i][:], tp[:], AF.Identity)
                            filler.append(unit)

                    def queue_proj_round(which, f):
                        # one t=3 projection output (k / q-head f / v): 16
                        # accumulating matmuls into the single spare PSUM bank,
                        # split into 4 filler units; the last unit evacuates.
                        cell = {}
                        for kc in range(KT // 4):
                            def unit(alt, kc=kc, which=which, f=f, cell=cell):
                                if kc == 0:
                                    cell["ps"] = prps.tile([128, 512], F32, name="prtile", tag="pr")
                                ps = cell["ps"]
                                for kk in range(4):
                                    k = 4 * kc + kk
                                    if which == "k":
                                        w = wk_sb[:, k, :]
                                    elif which == "v":
                                        w = wv_sb[:, k, :]
                                    else:
                                        w = wq_sb[:, k, f * 128:(f + 1) * 128]
                                    nc.tensor.matmul(ps[:], w, ht3[kc][:, kk, :],
                                                     start=(k == 0), stop=(k == KT - 1))
                                if kc == KT // 4 - 1:
                                    if which == "k":
                                        nc.scalar.activation(kt_sb[t3][:], ps[:], AF.Identity, bias=bk_sb[:])
                                    elif which == "v":
                                        vtr = vtrp.tile([128, 512], F16, name="vtr3", tag="vtr")
                                        vtr_tiles[t3] = vtr
                                        nc.scalar.activation(vtr[:], ps[:], AF.Identity, bias=bv_sb[:])
                                    else:
                                        nc.scalar.activation(qt_sb[f][t3][:], ps[:], AF.Identity, bias=bq_sb[:, f:f + 1])
                            filler.append(unit)

                    # filler supply, in deadline order: t3 K projection first
                    # (its inputs are ready before the phase-1 tail, unlike the
                    # vtrans units), v8-11 transposes (j2 needs them), t3 Q
                    # (rope3 runs during j2), t3 V + transposes (j3), strips.
                    queue_proj_round("k", 0)
                    queue_vtrans(2)
                    for f in range(HPC):
                        queue_proj_round("q", f)
                    queue_proj_round("v", 0)
                    queue_vtrans(3)

                    # j=0 first: its inputs (kt0, qt[h][0], v0..3) are roped and
                    # transposed earliest, so attention overlaps the projection
                    # tail. Later j's run with filler drained in.
                    for h in range(HPC):
                        attn_tile(h, 0, drain=drain)
                    queue_strip(0)
                    for h in range(HPC):
                        attn_tile(h, 1, drain=drain)
                    queue_strip(1)
                    for h in range(HPC):
                        a = attn_tile(h, 2, drain=drain)
                        # last tok tile's RoPE, spread between the j=2 tiles so
                        # the in-order DVE never blocks j1/j2 mask work on it
                        rope_inplace(t3, qt_sb[h][t3], after=a)
                        if h == HPC - 1:
                            rope_inplace(t3, kt_sb[t3], after=a)
                    queue_strip(2)
                    for h in range(HPC):
                        attn_tile(h, 3, drain=drain)
                    queue_strip(3)
                    drain(len(filler), alt=True)


            for pid in range(npasses):
                if pid > 0:
                    tc.strict_bb_all_engine_barrier()
                emit(pid)

    nc.compile()
    _PROGRAM_CACHE[key] = nc
    return nc


def build_in_maps(hidden_states, positions, Wq, bq, Wk, bk, Wv, bv, Wo):
    hidden_states = np.asarray(hidden_states, dtype=np.float32)
    positions = np.asarray(positions)
    Wq = np.asarray(Wq, dtype=np.float16)
    Wk = np.asarray(Wk, dtype=np.float16)
    Wv = np.asarray(Wv, dtype=np.float16)
    Wo = np.asarray(Wo, dtype=np.float16)
    bq = np.asarray(bq, dtype=np.float32)
    bk = np.asarray(bk, dtype=np.float32)
    bv = np.asarray(bv, dtype=np.float32)

    inv_freq = (1.0 / (THETA ** (np.arange(0, HD, 2, dtype=np.float32) / HD))).astype(np.float32)
    freqs = positions.astype(np.float32)[:, None] * inv_freq[None, :]      # [S, 64]
    cos_h = np.cos(freqs).T.astype(np.float32)                              # [64, S]
    sin_h = np.sin(freqs).T.astype(np.float32)
    cosT = np.ascontiguousarray(np.concatenate([cos_h, cos_h], axis=0)).astype(np.float16)   # [128, S]
    sinT = np.ascontiguousarray(np.concatenate([-sin_h, sin_h], axis=0)).astype(np.float16)  # [128, S]

    r = np.arange(128)[:, None]
    c = np.arange(128)[None, :]
    mask = (c >= r).astype(np.float16)
    ones = np.ones((128, 128), dtype=np.float16)
    eye = np.eye(128, dtype=np.float16)

    hidT = [np.ascontiguousarray(hidden_states[g].T.astype(np.float16)) for g in range(B)]

    in_maps = []
    for core in range(NCORES):
        g, t = core // TP, core % TP
        fs = slice(512 * t, 512 * (t + 1))
        ks = slice(128 * t, 128 * (t + 1))
        def pack(w):
            # [H, C] -> partition-major [128, KT*C] matching the device
            # rearrange "p (a m) -> p a m"
            c = w.shape[1]
            return np.ascontiguousarray(
                w.reshape(KT, 128, c).transpose(1, 0, 2).reshape(128, KT * c))
        in_maps.append({
            "hidT": hidT[g],
            "wq": pack(Wq[:, fs]),
            "wk": pack(Wk[:, ks]),
            "wv": pack(Wv[:, ks]),
            "wo": np.ascontiguousarray(Wo[fs, :]),
            "bq": np.ascontiguousarray(bq[fs].reshape(HPC, HD).T),
            "bk": np.ascontiguousarray(bk[ks].reshape(HD, 1)),
            "bv": np.ascontiguousarray(bv[ks].reshape(HD, 1)),
            "cosT": cosT,
            "sinT": sinT,
            "mask": mask,
            "ones": ones,
            "eye": eye,
        })
    return in_maps


def assemble(results):
    out = np.empty((B, S, H), dtype=np.float32)
    for g in range(B):
        acc = results[TP * g]["out"].astype(np.float32)
        for t in range(1, TP):
            acc += results[TP * g + t]["out"].astype(np.float32)
        out[g] = acc
    return out


def kernel(**inputs) -> np.ndarray:
    nc = build_program()
    in_maps = build_in_maps(**inputs)
    res = bass_utils.run_bass_kernel_spmd(nc, in_maps, list(range(NCORES)))
    return assemble(res.results)


# revision 31
# speedup vs baseline: 3.1163x; 3.1163x over previous
"""MiMo audio attention (B=2, S=2048, H=2048, NH=16, NKV=4, HD=128) on 8 trn2 cores.

Sharding: TP over heads x DP over batch. Cores 0-3 own batch 0, cores 4-7 own
batch 1. Within a batch group, TP rank t owns query heads [4t, 4t+4) and KV
head t (GQA: q head g uses kv head g//4, so the 4 q heads of rank t all use kv
head t). Each core computes a full-width o_proj partial over its 512
attn-output features; the host sums the 4 partials per batch group (the
"all-reduce after o_proj" of the TP scheme, done at gather time).

Device layout strategy (per core):
  - everything is fp16 on SBUF/DRAM (PSUM accumulation stays fp32): matmuls
    run at the same 1 cycle/row as fp32r at ANY free size (no >=256-col
    constraint), DVE tensor ops get the 2x two-byte fast path, and all DMA
    bytes halve. fp16 quantization (~5e-4 relative) is far inside the 2e-2
    correctness envelope.
  - hidden is fed pre-transposed as hidT [H, S] so the QKV projections run
    with W as the stationary operand and produce Q^T/K^T/V^T [feat, tok].
  - RoPE is applied in the [feat, tok] layout: cos/sin tables [128, S] are
    host-precomputed; rotate_half becomes a 64-partition swap done with two
    SBUF->SBUF DMAs.
  - scores are computed transposed, S^T[k, q] = K^T_tile^T @ Q^T, so attn@V
    needs no transposes: out^T[d, q] = V_tile^T @ exp(S^T). The softmax
    denominator is NOT a per-tile ones-matmul: masked exp tiles are
    accumulated elementwise (DVE/Pool alternating) into a per-(head,q-tile)
    fp16 accumulator, and a single [128,128] ones-matmul per (h,j) reduces it
    across partitions. This removes ~13% of all TensorE work.
  - softmax uses no max-subtraction (scores are O(5); exp is safe) and the
    causal mask is a multiplicative [128,128] (c>=r) triangle applied only to
    diagonal 128-blocks, post-exp, with exact 128-granular column truncation.
"""

import numpy as np

import concourse.bass as bass
import concourse.mybir as mybir
import concourse.tile as tile
from concourse import bacc, bass_utils
from concourse.tile_rust import add_dep_helper

B, S, H = 2, 2048, 2048
NH, NKV, HD = 16, 4, 128
THETA = 10000.0
SCALE = HD ** -0.5

NCORES = 8
TP = 4                 # cores per batch group
HPC = NH // TP         # 4 query heads per core
KT = H // 128          # 16 contraction tiles for projections
TT = S // 512          # 4 token tiles of 512
ST = S // 128          # 16 token tiles of 128

F32 = mybir.dt.float32
F16 = mybir.dt.float16
AF = mybir.ActivationFunctionType

_PROGRAM_CACHE = {}


def build_program(npasses=1):
    key = ("nc", npasses)
    if key in _PROGRAM_CACHE:
        return _PROGRAM_CACHE[key]

    nc = bacc.Bacc("TRN2", target_bir_lowering=False, debug=False, num_devices=NCORES)

    hidT = nc.declare_dram_parameter("hidT", [H, S], F16, isOutput=False)
    # weights arrive host-packed partition-major ([128, KT*cols]) so every
    # weight DMA is a >=2KB-contiguous run (256B runs pay a 2x DMA latency
    # penalty)
    wq = nc.declare_dram_parameter("wq", [128, KT * HPC * HD], F16, isOutput=False)
    wk = nc.declare_dram_parameter("wk", [128, KT * HD], F16, isOutput=False)
    wv = nc.declare_dram_parameter("wv", [128, KT * HD], F16, isOutput=False)
    wo = nc.declare_dram_parameter("wo", [HPC * HD, H], F16, isOutput=False)
    bq = nc.declare_dram_parameter("bq", [HD, HPC], F32, isOutput=False)
    bk = nc.declare_dram_parameter("bk", [HD, 1], F32, isOutput=False)
    bv = nc.declare_dram_parameter("bv", [HD, 1], F32, isOutput=False)
    cosT = nc.declare_dram_parameter("cosT", [HD, S], F16, isOutput=False)
    sinT = nc.declare_dram_parameter("sinT", [HD, S], F16, isOutput=False)
    mask = nc.declare_dram_parameter("mask", [128, 128], F16, isOutput=False)
    ones = nc.declare_dram_parameter("ones", [128, 128], F16, isOutput=False)
    eye = nc.declare_dram_parameter("eye", [128, 128], F16, isOutput=False)
    out_d = nc.declare_dram_parameter("out", [S, H], F16, isOutput=True)

    hidT_r = hidT.ap().rearrange("(a p) m -> p a m", p=128)   # [128, 16, 2048]
    wq_r = wq.ap().rearrange("p (a m) -> p a m", a=KT)
    wk_r = wk.ap().rearrange("p (a m) -> p a m", a=KT)
    wv_r = wv.ap().rearrange("p (a m) -> p a m", a=KT)

    with tile.TileContext(nc) as tc:
        with (
            tc.tile_pool(name="consts", bufs=1) as consts,
            tc.tile_pool(name="persist", bufs=1) as persist,
            tc.tile_pool(name="wts", bufs=1) as wts,
            tc.tile_pool(name="hid3", bufs=1) as hid3,
            tc.tile_pool(name="vtr", bufs=2) as vtrp,
            tc.tile_pool(name="expp", bufs=6) as expp,
            tc.tile_pool(name="accp", bufs=4) as accp,
            tc.tile_pool(name="recp", bufs=2) as recp,
            tc.tile_pool(name="trig", bufs=1) as trig,
            tc.tile_pool(name="stage", bufs=3) as stage,
        ):
            mask_sb = consts.tile([128, 128], F16)
            ones_sb = consts.tile([128, 128], F16)
            eye_sb = consts.tile([128, 128], F16)
            bq_sb = consts.tile([HD, HPC], F32)
            bk_sb = consts.tile([HD, 1], F32)
            bv_sb = consts.tile([HD, 1], F32)
            # const loads ride the idle Pool queue so the ACT queue's first DMAs
            # are the weight chunks the first matmuls wait on
            nc.gpsimd.dma_start(bq_sb[:], bq.ap())
            nc.gpsimd.dma_start(bk_sb[:], bk.ap())
            nc.gpsimd.dma_start(bv_sb[:], bv.ap())
            nc.gpsimd.dma_start(eye_sb[:], eye.ap())
            nc.gpsimd.dma_start(mask_sb[:], mask.ap())
            nc.gpsimd.dma_start(ones_sb[:], ones.ap())

            def emit(pid):
                # persistent activations, one tile per (tensor, tok-tile) so
                # cross-phase dependencies stay precise
                qt_sb = [[persist.tile([128, 512], F16, name=f"qt{h}_{t}", tag=f"qt{h}_{t}")
                          for t in range(TT)] for h in range(HPC)]
                kt_sb = [persist.tile([128, 512], F16, name=f"kt_{t}", tag=f"kt_{t}") for t in range(TT)]
                v_sb = [persist.tile([128, 128], F16, name=f"v_{i}", tag=f"v_{i}") for i in range(ST)]
                ao_sb = [[persist.tile([128, 512], F16, name=f"ao{h}_{t}", tag=f"ao{h}_{t}")
                          for t in range(TT)] for h in range(HPC)]

                cos_sb = trig.tile([HD, S], F16, name="cos_sb")
                sin_sb = trig.tile([HD, S], F16, name="sin_sb")

                def rope_inplace(t, dst, after=None):
                    """dst holds raw (biased) values for tok tile t; rotate in place.

                    `after`: optional instruction; adds a scheduler-only edge so
                    this chain is ordered behind it (keeps the in-order DVE FIFO
                    from blocking earlier-needed work behind this chain)."""
                    tok = bass.ds(t * 512, 512)
                    swp = stage.tile([128, 512], F16, tag="swp")
                    d0 = nc.gpsimd.dma_start(swp[0:64, :], dst[64:128, :])
                    d1 = nc.gpsimd.dma_start(swp[64:128, :], dst[0:64, :])
                    if after is not None:
                        add_dep_helper(d0.ins, after.ins, False, "delay last-tok rope")
                        add_dep_helper(d1.ins, after.ins, False, "delay last-tok rope")
                    m0 = nc.vector.tensor_mul(dst[:], dst[:], cos_sb[:, tok])
                    if after is not None:
                        add_dep_helper(m0.ins, after.ins, False, "delay last-tok rope")
                    nc.vector.tensor_mul(swp[:], swp[:], sin_sb[:, tok])
                    nc.vector.tensor_add(dst[:], dst[:], swp[:])

                # ---------------- phase 1: QKV projection + RoPE -----------------
                # Only tok tiles 0..TT-2 project here; the t=TT-1 projection is
                # deferred into the attention phase as PE filler work (see
                # phase 2), where attention's exp-rate otherwise leaves the
                # tensor engine idle.
                with (
                    tc.tile_pool(name=f"hidp{pid}", bufs=2) as hidp,
                    tc.tile_pool(name=f"ppsA{pid}", bufs=1, space=bass.MemorySpace.PSUM) as pps,
                    tc.tile_pool(name=f"ppsB{pid}", bufs=1, space=bass.MemorySpace.PSUM) as ppsB,
                ):
                    wq_sb = wts.tile([128, KT, HPC * HD], F16)
                    wk_sb = wts.tile([128, KT, HD], F16)
                    wv_sb = wts.tile([128, KT, HD], F16)

                    vtr_tiles = {}
                    for t in range(TT - 1):
                        tok = bass.ds(t * 512, 512)
                        q_ps = [pps.tile([128, 512], F32, name=f"qps{f}", tag=f"qps{f}") for f in range(HPC)]
                        k_ps = ppsB.tile([128, 512], F32, tag="kps")
                        v_ps = ppsB.tile([128, 512], F32, tag="vps")
                        # V^T -> V transposes for the PREVIOUS tok tile, emitted
                        # first (their inputs are long ready; copies go to ACT so
                        # they never queue behind RoPE work on the DVE)
                        if t > 0:
                            for i in range(4 * (t - 1), 4 * t):
                                tp = ppsB.tile([128, 128], F16, tag="vt", bufs=2)
                                nc.tensor.transpose(tp[:], vtr_tiles[t - 1][:, (i % 4) * 128:(i % 4 + 1) * 128], eye_sb[:])
                                nc.scalar.activation(v_sb[i][:], tp[:], AF.Identity)
                        for kc in range(KT // 4):       # 4 k-slices per DMA chunk
                            if t == 0:
                                nc.scalar.dma_start(wk_sb[:, 4 * kc:4 * (kc + 1), :],
                                                    wk_r[:, 4 * kc:4 * (kc + 1), :])
                                if kc == 0:
                                    # split the first wq chunk per head slice so
                                    # the first Q matmul starts after 128KB, not
                                    # 512KB, and before wv occupies the HWDGE
                                    for f in range(HPC):
                                        nc.scalar.dma_start(
                                            wq_sb[:, 0:4, f * 128:(f + 1) * 128],
                                            wq_r[:, 0:4, f * 128:(f + 1) * 128])
                                    nc.scalar.dma_start(wv_sb[:, 0:4, :], wv_r[:, 0:4, :])
                                else:
                                    nc.scalar.dma_start(wv_sb[:, 4 * kc:4 * (kc + 1), :],
                                                        wv_r[:, 4 * kc:4 * (kc + 1), :])
                                    nc.scalar.dma_start(wq_sb[:, 4 * kc:4 * (kc + 1), :],
                                                        wq_r[:, 4 * kc:4 * (kc + 1), :])
                            ht = hidp.tile([128, 4, 512], F16)
                            if t == 0 and kc == 0:
                                # split the first chunk so the first matmuls
                                # start after half the transfer
                                nc.sync.dma_start(ht[:, 0:2, :], hidT_r[:, 0:2, tok])
                                nc.sync.dma_start(ht[:, 2:4, :], hidT_r[:, 2:4, tok])
                            else:
                                nc.sync.dma_start(ht[:], hidT_r[:, 4 * kc:4 * (kc + 1), tok])
                            for kk in range(4):
                                k = 4 * kc + kk
                                st, sp = (k == 0), (k == KT - 1)
                                nc.tensor.matmul(k_ps[:], wk_sb[:, k, :], ht[:, kk, :], start=st, stop=sp)
                                for f in range(HPC):
                                    nc.tensor.matmul(q_ps[f][:], wq_sb[:, k, f * 128:(f + 1) * 128], ht[:, kk, :], start=st, stop=sp)
                                nc.tensor.matmul(v_ps[:], wv_sb[:, k, :], ht[:, kk, :], start=st, stop=sp)
                        if t == 0:
                            nc.scalar.dma_start(cos_sb[:], cosT.ap())
                            nc.scalar.dma_start(sin_sb[:], sinT.ap())

                        # pass 1: evacuate all six PSUM banks. Mid-phase tiles
                        # go all-ACT (keeps DVE free for RoPE); the LAST tile's
                        # six evacs all land after its final matmul and gate the
                        # phase-2 PSUM pool handover, so split them ACT/DVE to
                        # halve the serial drain.
                        last_t = (t == TT - 2)
                        for f in range(HPC):
                            if last_t and f % 2 == 1:
                                nc.vector.tensor_scalar_add(qt_sb[f][t][:], q_ps[f][:], bq_sb[:, f:f + 1])
                            else:
                                nc.scalar.activation(qt_sb[f][t][:], q_ps[f][:], AF.Identity, bias=bq_sb[:, f:f + 1])
                        vtr = vtrp.tile([128, 512], F16, tag="vtr")
                        vtr_tiles[t] = vtr
                        nc.scalar.activation(vtr[:], v_ps[:], AF.Identity, bias=bv_sb[:])
                        if last_t:
                            nc.vector.tensor_scalar_add(kt_sb[t][:], k_ps[:], bk_sb[:])
                        else:
                            nc.scalar.activation(kt_sb[t][:], k_ps[:], AF.Identity, bias=bk_sb[:])

                        # pass 2: RoPE in place on Q heads and K
                        for f in range(HPC):
                            rope_inplace(t, qt_sb[f][t])
                        rope_inplace(t, kt_sb[t])

                # ---------------- phase 2: attention + o_proj ---------------------
                # Attention's inner loop is ACT-bound (one exp per 128-k tile at
                # ~680ns vs ~340ns of PE work), so PE-side FILLER work drains
                # into the attention instruction stream: first the deferred
                # t=TT-1 projection (one PSUM bank, one output at a time), then
                # o_proj 512-col chunks as their ao strips complete. Supply
                # matches demand: proj units cover j0+j1, strip0 covers j2,
                # strips 1-2 cover j3, strip3 is the PE-only tail.
                with (
                    tc.tile_pool(name=f"wo_p{pid}", bufs=1) as wo_p,
                    tc.tile_pool(name=f"outp{pid}", bufs=3) as outp,
                    tc.tile_pool(name=f"scps{pid}", bufs=2, space=bass.MemorySpace.PSUM) as scps,
                    tc.tile_pool(name=f"oups{pid}", bufs=2, space=bass.MemorySpace.PSUM) as oups,
                    tc.tile_pool(name=f"smps{pid}", bufs=1, space=bass.MemorySpace.PSUM) as smps,
                    tc.tile_pool(name=f"opps{pid}", bufs=2, space=bass.MemorySpace.PSUM) as opps,
                    tc.tile_pool(name=f"prps{pid}", bufs=1, space=bass.MemorySpace.PSUM) as prps,
                ):
                    t3 = TT - 1
                    # t=3 hidden chunks + wo ride the now-idle SP queue
                    ht3 = []
                    for kc in range(KT // 4):
                        h3 = hid3.tile([128, 4, 512], F16, tag=f"t3c{kc}")
                        nc.sync.dma_start(h3[:], hidT_r[:, 4 * kc:4 * (kc + 1), bass.ds(t3 * 512, 512)])
                        ht3.append(h3)
                    wo_sb = wo_p.tile([128, HPC, H], F16)
                    for k in range(HPC):
                        nc.sync.dma_start(wo_sb[:, k, :],
                                          wo.ap().rearrange("(t p) m -> p t m", p=128)[:, k, :])

                    def attn_tile(h, j, drain=None):
                        ou_ps = oups.tile([128, 512], F32, tag="ou")
                        acc = accp.tile([128, 512], F16, tag="acc")
                        last = 4 * j + 3
                        pend = None  # software-pipeline: consumer MMs trail by one i
                        for i in range(last + 1):
                            d = i - 4 * j
                            c0 = 0 if d < 0 else 128 * d
                            sc_ps = scps.tile([128, 512], F32, tag="sc")
                            nc.tensor.matmul(
                                sc_ps[:, c0:512],
                                kt_sb[i // 4][:, (i % 4) * 128:(i % 4 + 1) * 128],
                                qt_sb[h][j][:, c0:512],
                                start=True, stop=True,
                            )
                            ex = expp.tile([128, 512], F16)
                            nc.scalar.activation(ex[:, c0:512], sc_ps[:, c0:512], AF.Exp, scale=SCALE)
                            if d >= 0:
                                # causal mask on the diagonal 128-block
                                nc.vector.tensor_mul(
                                    ex[:, c0:c0 + 128],
                                    ex[:, c0:c0 + 128],
                                    mask_sb[:],
                                )
                            # accumulate exp for the softmax denominator
                            if i == 0:
                                nc.vector.tensor_copy(acc[:], ex[:])
                            else:
                                nc.vector.tensor_add(acc[:, c0:512], acc[:, c0:512], ex[:, c0:512])
                            if pend is not None:
                                pex, pc0, pi = pend
                                nc.tensor.matmul(ou_ps[:, pc0:512], v_sb[pi][:], pex[:, pc0:512],
                                                 start=(pi == 0), stop=False)
                            pend = (ex, c0, i)
                            if drain is not None and i % 2 == 1:
                                drain(1)
                        pex, pc0, pi = pend
                        nc.tensor.matmul(ou_ps[:, pc0:512], v_sb[pi][:], pex[:, pc0:512],
                                         start=(pi == 0), stop=True)
                        sm_ps = smps.tile([128, 512], F32, tag="sm")
                        nc.tensor.matmul(sm_ps[:], ones_sb[:], acc[:], start=True, stop=True)
                        rec = recp.tile([128, 512], F32)
                        nc.vector.reciprocal_approx_fast(rec[:], sm_ps[:])
                        return nc.vector.tensor_mul(ao_sb[h][j][:], ou_ps[:], rec[:])

                    # --- filler machinery: closures drained into attention ---
                    from collections import deque
                    filler = deque()

                    def emit_chunk(m, n, alt=False):
                        ot = outp.tile([128, 512], F16, tag="ot")
                        ps = opps.tile([128, 512], F32, tag="op")
                        for k in range(HPC):
                            nc.tensor.matmul(
                                ps[:],
                                ao_sb[k][m // 4][:, (m % 4) * 128:(m % 4 + 1) * 128],
                                wo_sb[:, k, n * 512:(n + 1) * 512],
                                start=(k == 0), stop=(k == HPC - 1),
                            )
                        # during attention the ACT queue must stay exp-only, so
                        # filler evacuations go to DVE; tail chunks alternate
                        # engines and output-DMA queues (SP/ACT) to halve the
                        # end-of-kernel queue drain
                        if alt and (m + n) % 2 == 0:
                            nc.scalar.activation(ot[:], ps[:], AF.Identity)
                        else:
                            nc.vector.tensor_copy(ot[:], ps[:])
                        nc.sync.dma_start(out_d.ap()[m * 128:(m + 1) * 128, n * 512:(n + 1) * 512],
                                          ot[:])

                    def drain(k, alt=False):
                        for _ in range(k):
                            if filler:
                                filler.popleft()(alt)

                    def queue_strip(j):
                        for m in range(4 * j, 4 * j + 4):
                            for n in range(TT):
                                filler.append(lambda alt, m=m, n=n: emit_chunk(m, n, alt))

                    def queue_vtrans(t):
                        for i in range(4 * t, 4 * t + 4):
                            def unit(alt, i=i, t=t):
                                tp = opps.tile([128, 128], F16, name="vtp", tag="op")
                                nc.tensor.transpose(tp[:], vtr_tiles[t][:, (i % 4) * 128:(i % 4 + 1) * 128], eye_sb[:])
                                nc.scalar.activation(v_sb[i][:], tp[:], AF.Identity)
                            filler.append(unit)

                    def queue_proj_round(which, f):
                        # one t=3 projection output (k / q-head f / v): 16
                        # accumulating matmuls into the single spare PSUM bank,
                        # split into 4 filler units; the last unit evacuates.
                        cell = {}
                        for kc in range(KT // 4):
                            def unit(alt, kc=kc, which=which, f=f, cell=cell):
                                if kc == 0:
                                    cell["ps"] = prps.tile([128, 512], F32, name="prtile", tag="pr")
                                ps = cell["ps"]
                                for kk in range(4):
                                    k = 4 * kc + kk
                                    if which == "k":
                                        w = wk_sb[:, k, :]
                                    elif which == "v":
                                        w = wv_sb[:, k, :]
                                    else:
                                        w = wq_sb[:, k, f * 128:(f + 1) * 128]
                                    nc.tensor.matmul(ps[:], w, ht3[kc][:, kk, :],
                                                     start=(k == 0), stop=(k == KT - 1))
                                if kc == KT // 4 - 1:
                                    if which == "k":
                                        nc.scalar.activation(kt_sb[t3][:], ps[:], AF.Identity, bias=bk_sb[:])
                                    elif which == "v":
                                        vtr = vtrp.tile([128, 512], F16, name="vtr3", tag="vtr")
                                        vtr_tiles[t3] = vtr
                                        nc.scalar.activation(vtr[:], ps[:], AF.Identity, bias=bv_sb[:])
                                    else:
                                        nc.scalar.activation(qt_sb[f][t3][:], ps[:], AF.Identity, bias=bq_sb[:, f:f + 1])
                            filler.append(unit)

                    # filler supply, in deadline order: t3 K projection first
                    # (its inputs are ready before the phase-1 tail, unlike the
                    # vtrans units), v8-11 transposes (j2 needs them), t3 Q
                    # (rope3 runs during j2), t3 V + transposes (j3), strips.
                    queue_proj_round("k", 0)
                    queue_vtrans(2)
                    for f in range(HPC):
                        queue_proj_round("q", f)
                    queue_proj_round("v", 0)
                    queue_vtrans(3)

                    # j=0 first: its inputs (kt0, qt[h][0], v0..3) are roped and
                    # transposed earliest, so attention overlaps the projection
                    # tail. Later j's run with filler drained in.
                    for h in range(HPC):
                        attn_tile(h, 0, drain=drain)
                    queue_strip(0)
                    for h in range(HPC):
                        attn_tile(h, 1, drain=drain)
                    queue_strip(1)
                    for h in range(HPC):
                        a = attn_tile(h, 2, drain=drain)
                        # last tok tile's RoPE, spread between the j=2 tiles so
                        # the in-order DVE never blocks j1/j2 mask work on it
                        rope_inplace(t3, qt_sb[h][t3], after=a)
                        if h == HPC - 1:
                            rope_inplace(t3, kt_sb[t3], after=a)
                    queue_strip(2)
                    for h in range(HPC):
                        attn_tile(h, 3, drain=drain)
                    queue_strip(3)
                    drain(len(filler), alt=True)


            for pid in range(npasses):
                if pid > 0:
                    tc.strict_bb_all_engine_barrier()
                emit(pid)

    nc.compile()
    _PROGRAM_CACHE[key] = nc
    return nc


def build_in_maps(hidden_states, positions, Wq, bq, Wk, bk, Wv, bv, Wo):
    hidden_states = np.asarray(hidden_states, dtype=np.float32)
    positions = np.asarray(positions)
    Wq = np.asarray(Wq, dtype=np.float16)
    Wk = np.asarray(Wk, dtype=np.float16)
    Wv = np.asarray(Wv, dtype=np.float16)
    Wo = np.asarray(Wo, dtype=np.float16)
    bq = np.asarray(bq, dtype=np.float32)
    bk = np.asarray(bk, dtype=np.float32)
    bv = np.asarray(bv, dtype=np.float32)

    inv_freq = (1.0 / (THETA ** (np.arange(0, HD, 2, dtype=np.float32) / HD))).astype(np.float32)
    freqs = positions.astype(np.float32)[:, None] * inv_freq[None, :]      # [S, 64]
    cos_h = np.cos(freqs).T.astype(np.float32)                              # [64, S]
    sin_h = np.sin(freqs).T.astype(np.float32)
    cosT = np.ascontiguousarray(np.concatenate([cos_h, cos_h], axis=0)).astype(np.float16)   # [128, S]
    sinT = np.ascontiguousarray(np.concatenate([-sin_h, sin_h], axis=0)).astype(np.float16)  # [128, S]

    r = np.arange(128)[:, None]
    c = np.arange(128)[None, :]
    mask = (c >= r).astype(np.float16)
    ones = np.ones((128, 128), dtype=np.float16)
    eye = np.eye(128, dtype=np.float16)

    hidT = [np.ascontiguousarray(hidden_states[g].T.astype(np.float16)) for g in range(B)]

    in_maps = []
    for core in range(NCORES):
        g, t = core // TP, core % TP
        fs = slice(512 * t, 512 * (t + 1))
        ks = slice(128 * t, 128 * (t + 1))
        def pack(w):
            # [H, C] -> partition-major [128, KT*C] matching the device
            # rearrange "p (a m) -> p a m"
            c = w.shape[1]
            return np.ascontiguousarray(
                w.reshape(KT, 128, c).transpose(1, 0, 2).reshape(128, KT * c))
        in_maps.append({
            "hidT": hidT[g],
            "wq": pack(Wq[:, fs]),
            "wk": pack(Wk[:, ks]),
            "wv": pack(Wv[:, ks]),
            "wo": np.ascontiguousarray(Wo[fs, :]),
            "bq": np.ascontiguousarray(bq[fs].reshape(HPC, HD).T),
            "bk": np.ascontiguousarray(bk[ks].reshape(HD, 1)),
            "bv": np.ascontiguousarray(bv[ks].reshape(HD, 1)),
            "cosT": cosT,
            "sinT": sinT,
            "mask": mask,
            "ones": ones,
            "eye": eye,
        })
    return in_maps


def assemble(results):
    out = np.empty((B, S, H), dtype=np.float32)
    for g in range(B):
        acc = results[TP * g]["out"].astype(np.float32)
        for t in range(1, TP):
            acc += results[TP * g + t]["out"].astype(np.float32)
        out[g] = acc
    return out


def kernel(**inputs) -> np.ndarray:
    nc = build_program()
    in_maps = build_in_maps(**inputs)
    res = bass_utils.run_bass_kernel_spmd(nc, in_maps, list(range(NCORES)))
    return assemble(res.results)
